# revision 16
# baseline (speedup 1.0000x reference)
"""Multi-head self-attention Bass kernel for Trainium2, 8 NeuronCores.

Problem: B=4, S=2048, D=1024, H=16 heads (dh=64), causal mask, fp32.

Sharding: core c -> batch b = c//2, head-group g = c%2 (8 heads each).
Data-parallel over B, tensor-parallel over heads; out-proj is row-parallel
with the partial-sum reduction done on the host (plus all output biases,
which fold into a single per-feature vector added on the host).

Per-core kernel (all matmuls in float32r = full-speed reduced-precision fp32):
  phase A: qkT = (Wqk x^T) [f,s layout] + bias;  V = x Wv^T [s,f layout]
  phase B: per (q-block 512, head): scoresT[k,q] = kT^T qT; P = exp(s/8);
           multiply diagonal blocks by the keep-mask; ctxT[f,q] = V_ext^T P
           with a ones column appended to V giving the softmax denominator;
           normalize via DVE multiply with a DMA-broadcast reciprocal row.
  phase C: partial_out[s,d] = ctxT^T WoT, streamed to DRAM.
Host: out[b] = partial[2b] + partial[2b+1] + (b_out + w_out @ b_v).
"""

import sys

sys.path.insert(0, "/opt/trn_rl_repo")

import ml_dtypes
import numpy as np

import concourse.bass as bass  # noqa: F401
import concourse.mybir as mybir
import concourse.tile as tile
from concourse import bacc
from concourse.bass_utils import run_bass_kernel_spmd

B = 4
S = 2048
D = 1024
H = 16
DH = 64
N_CORES = 8
HPC = H // 2          # heads per core = 8
FL = HPC * DH         # local feature width = 512
QB = 512              # q-block width
KB = 128              # k-block width
ND = D // 128         # d-blocks = 8

F32 = mybir.dt.float32
F32R = mybir.dt.float32r
BF16 = mybir.dt.bfloat16
SCALE = 1.0 / np.sqrt(DH)

_COMPILED = {}


def build_nc(s: int = S, body_reps: int = 1):
    """Build the SPMD Bass program for sequence length s. body_reps>1
    repeats the compute body (for marginal wall-clock measurement)."""
    nqb = s // QB
    nkb_all = s // KB
    nst = s // 128
    NFB = 2 * FL // 128   # 8 f-tiles of qkT (4 q + 4 k)

    nc = bacc.Bacc("TRN2", target_bir_lowering=False, debug=False,
                   num_devices=N_CORES)

    xT = nc.declare_dram_parameter("xT", [D, s], F32R, isOutput=False)
    wqkT = nc.declare_dram_parameter("wqkT", [D, 2 * FL], F32R, isOutput=False)
    wvT = nc.declare_dram_parameter("wvT", [D, FL], F32R, isOutput=False)
    b_qk = nc.declare_dram_parameter("b_qk", [128, NFB], F32, isOutput=False)
    woT = nc.declare_dram_parameter("woT", [FL, D], F32R, isOutput=False)
    band = nc.declare_dram_parameter("band", [nqb * QB, QB], BF16, isOutput=False)
    out = nc.declare_dram_parameter("out", [s, D], F32, isOutput=True)

    with tile.TileContext(nc) as tc:
        with (
            tc.tile_pool(name="persist", bufs=1) as pp,
            tc.tile_pool(name="mmpsum", bufs=2, space="PSUM") as psA,
            tc.tile_pool(name="scpsum", bufs=2, space="PSUM") as psS,
            tc.tile_pool(name="ctxpsum", bufs=2, space="PSUM") as psC,
        ):
            qkT_sb = pp.tile([128, NFB, s], F32R)
            v_ext = pp.tile([128, nst, HPC, DH + 1], F32R)
            ones_sb = pp.tile([128, HPC], F32)
            nc.vector.memset(ones_sb[:], 1.0)
            onesf = pp.tile([128, 64], F32)
            nc.vector.memset(onesf[:], 1.0)
            ones64 = pp.tile([128, 64], F32R)
            nc.vector.tensor_copy(ones64[:], onesf[:])

            for rep in range(body_reps):
                # ---------------- phase A: projections ----------------
                with tc.tile_pool(name=f"phA_{rep}", bufs=1) as pa, \
                     tc.tile_pool(name=f"wqk_{rep}", bufs=2) as pw:
                    xT_sb = pa.tile([128, ND, s], F32R, tag="xT")
                    xv = xT.ap().rearrange("(a p) s -> p a s", p=128)
                    for d in range(ND):
                        nc.sync.dma_start(xT_sb[:, d, :], xv[:, d, :])
                    wvT_sb = pa.tile([128, ND, FL], F32R, tag="wvT")
                    nc.sync.dma_start(
                        wvT_sb[:], wvT.ap().rearrange("(a p) f -> p a f", p=128))
                    bqk_sb = pa.tile([128, NFB], F32, tag="bqk")
                    nc.sync.dma_start(bqk_sb[:], b_qk.ap())

                    wqkv = wqkT.ap().rearrange("(a p) f -> p a f", p=128)
                    for fb in range(NFB):
                        wt = pw.tile([128, ND, 128], F32R, tag="wqk")
                        nc.sync.dma_start(wt[:], wqkv[:, :, 128 * fb:128 * fb + 128])
                        for sbk in range(nqb):
                            ps = psA.tile([128, QB], F32, tag="mm")
                            for d in range(ND):
                                nc.tensor.matmul(
                                    ps[:], wt[:, d, :],
                                    xT_sb[:, d, QB * sbk:QB * (sbk + 1)],
                                    start=(d == 0), stop=(d == ND - 1))
                            nc.vector.tensor_scalar_add(
                                qkT_sb[:, fb, QB * sbk:QB * (sbk + 1)], ps[:],
                                bqk_sb[:, fb:fb + 1])

                    for st in range(nst):
                        ps = psA.tile([128, FL], F32, tag="mm")
                        for d in range(ND):
                            nc.tensor.matmul(
                                ps[:], xT_sb[:, d, 128 * st:128 * (st + 1)],
                                wvT_sb[:, d, :],
                                start=(d == 0), stop=(d == ND - 1))
                        nc.vector.tensor_copy(
                            v_ext[:, st, :, 0:DH],
                            ps[:].rearrange("p (h e) -> p h e", h=HPC))
                        nc.vector.tensor_copy(v_ext[:, st, :, DH], ones_sb[:])

                # ---------------- phases B+C: attention + out-proj ----------
                with tc.tile_pool(name=f"phB_{rep}", bufs=1) as pb, \
                     tc.tile_pool(name=f"pP_{rep}", bufs=5) as pP, \
                     tc.tile_pool(name=f"pN_{rep}", bufs=2) as pN, \
                     tc.tile_pool(name=f"pctx_{rep}", bufs=2) as pctx, \
                     tc.tile_pool(name=f"pout_{rep}", bufs=2) as pout:
                    band_sb = pb.tile([128, nqb * 4, QB], BF16, tag="band")
                    nc.sync.dma_start(
                        band_sb[:],
                        band.ap().rearrange("(i k p) q -> p (i k) q", p=128, k=4))
                    woT_sb = pb.tile([128, FL // 128, D], F32R, tag="woT")
                    nc.sync.dma_start(
                        woT_sb[:], woT.ap().rearrange("(a p) d -> p a d", p=128))

                    for qb in range(nqb):
                        ctx_sb = pctx.tile([128, FL // 128, QB], F32R, tag="ctx")
                        nkb = 4 * (qb + 1)
                        for hp in range(HPC // 2):
                            # head pair (2hp, 2hp+1): h0 lives at partitions
                            # 0-63 of f-tile hp, h1 at 64-127 — the two K=64
                            # scores matmuls target disjoint PE row groups and
                            # run concurrently.
                            h0, h1 = 2 * hp, 2 * hp + 1
                            kfb = 4 + hp
                            qT0 = qkT_sb[0:64, hp, QB * qb:QB * (qb + 1)]
                            qT1 = qkT_sb[64:128, hp, QB * qb:QB * (qb + 1)]
                            pc0 = psC.tile([DH + 1, QB], F32, tag="ctxp")
                            pc1 = psC.tile([DH + 1, QB], F32, tag="ctxp")
                            for kb in range(nkb):
                                # on diagonal blocks, columns q' < off are fully
                                # masked: skip them in scores/exp/PV, and apply
                                # the keep-mask only on the mixed 128-wide square.
                                diag = kb >= 4 * qb
                                off = 128 * (kb - 4 * qb) if diag else 0
                                ps = psS.tile([128, 2, QB], F32, tag="sc")
                                kcol = slice(128 * kb, 128 * (kb + 1))
                                nc.tensor.matmul(
                                    ps[:, 0, off:], qkT_sb[0:64, kfb, kcol],
                                    qT0[:, off:], start=True, stop=True)
                                nc.tensor.matmul(
                                    ps[:, 1, off:], qkT_sb[64:128, kfb, kcol],
                                    qT1[:, off:], start=True, stop=True)
                                pt = pP.tile([128, 2, QB], F32R, tag="P")
                                nc.scalar.activation(
                                    pt[:, :, off:], ps[:, :, off:],
                                    mybir.ActivationFunctionType.Exp,
                                    scale=float(SCALE))
                                if diag:
                                    nc.vector.tensor_mul(
                                        pt[:, :, off:off + 128],
                                        pt[:, :, off:off + 128],
                                        band_sb[:, 4 * qb + (kb - 4 * qb), None,
                                                off:off + 128]
                                        .to_broadcast((128, 2, 128)))
                                nc.tensor.matmul(
                                    pc0[:, off:], v_ext[:, kb, h0, :],
                                    pt[:, 0, off:],
                                    start=(kb == 0), stop=(kb == nkb - 1))
                                nc.tensor.matmul(
                                    pc1[:, off:], v_ext[:, kb, h1, :],
                                    pt[:, 1, off:],
                                    start=(kb == 0), stop=(kb == nkb - 1))
                            # normalize: reciprocal of the denominator row,
                            # PE ones-outer-product broadcast, multiply.
                            for j, pc in ((0, pc0), (1, pc1)):
                                rec = pN.tile([128, QB], F32R, tag="rec")
                                with nc.allow_low_precision(
                                        reason="f32r recip feeds PE broadcast"):
                                    nc.vector.reciprocal(rec[64:65, :],
                                                         pc[DH:DH + 1, :])
                                pbc = psA.tile([64, QB], F32, tag="mm")
                                nc.tensor.matmul(
                                    pbc[:], ones64[64:65, :], rec[64:65, :],
                                    start=True, stop=True)
                                bc = pN.tile([64, QB], F32, tag="bc")
                                nc.scalar.activation(
                                    bc[:], pbc[:],
                                    mybir.ActivationFunctionType.Copy)
                                if j == 0:
                                    nc.vector.tensor_mul(
                                        ctx_sb[0:64, hp, :], pc[0:DH, :], bc[:])
                                else:
                                    tmp = pN.tile([64, QB], F32R, tag="tmp")
                                    nc.vector.tensor_mul(tmp[:], pc[0:DH, :],
                                                         bc[:])
                                    nc.sync.dma_start(ctx_sb[64:128, hp, :],
                                                      tmp[:])

                        # out-proj for this q-block
                        for st4 in range(QB // 128):
                            ot = pout.tile([128, D], F32, tag="ot")
                            for dh2 in range(2):
                                po = psA.tile([128, 512], F32, tag="mm")
                                for fb2 in range(FL // 128):
                                    nc.tensor.matmul(
                                        po[:],
                                        ctx_sb[:, fb2, 128 * st4:128 * (st4 + 1)],
                                        woT_sb[:, fb2, 512 * dh2:512 * (dh2 + 1)],
                                        start=(fb2 == 0), stop=(fb2 == 3))
                                nc.vector.tensor_copy(
                                    ot[:, 512 * dh2:512 * (dh2 + 1)], po[:])
                            st = 4 * qb + st4
                            nc.sync.dma_start(
                                out[128 * st:128 * (st + 1), :], ot[:])
    nc.finalize()
    return nc


def prep_in_maps(x, mask, w_qkv, b_qkv, w_out, s: int = S):
    nqb = s // QB
    m = np.asarray(mask)[0, 0]
    band = np.empty((nqb * QB, QB), ml_dtypes.bfloat16)
    for i in range(nqb):
        band[QB * i:QB * (i + 1)] = (~m[QB * i:QB * (i + 1),
                                        QB * i:QB * (i + 1)]).T
    in_maps = []
    for c in range(N_CORES):
        b, g = c // 2, c % 2
        wq = w_qkv[FL * g:FL * (g + 1)]
        wk = w_qkv[D + FL * g:D + FL * (g + 1)]
        wv = w_qkv[2 * D + FL * g:2 * D + FL * (g + 1)]
        bq = b_qkv[FL * g:FL * (g + 1)]
        bk = b_qkv[D + FL * g:D + FL * (g + 1)]
        bqk = np.concatenate([bq, bk]).reshape(2 * FL // 128, 128).T
        in_maps.append({
            "xT": np.ascontiguousarray(x[b].T),
            "wqkT": np.ascontiguousarray(np.concatenate([wq, wk], 0).T),
            "wvT": np.ascontiguousarray(wv.T),
            "b_qk": np.ascontiguousarray(bqk),
            "woT": np.ascontiguousarray(w_out[:, FL * g:FL * (g + 1)].T),
            "band": band,
        })
    return in_maps


def assemble(results, b_qkv, w_out, b_out, s: int = S):
    bv = b_qkv[2 * D:]
    bias_full = b_out + w_out @ bv
    outp = np.empty((B, s, D), np.float32)
    for b in range(B):
        outp[b] = (results[2 * b]["out"] + results[2 * b + 1]["out"]
                   + bias_full[None, :])
    return outp


def _make_runner(nc, in_maps):
    """Persistent PJRT runner: trace/compile once, reuse the executable."""
    import jax
    from jax.sharding import Mesh, PartitionSpec, NamedSharding
    from jax.experimental.shard_map import shard_map
    from concourse import bass2jax
    from concourse.bass2jax import _bass_exec_p, partition_id_tensor

    bass2jax.install_neuronx_cc_hook()
    partition_name = nc.partition_id_tensor.name if nc.partition_id_tensor else None
    in_names, out_names, out_avals, zero_outs = [], [], [], []
    for alloc in nc.m.functions[0].allocations:
        if not isinstance(alloc, mybir.MemoryLocationSet):
            continue
        name = alloc.memorylocations[0].name
        if alloc.kind == "ExternalInput":
            if name != partition_name:
                in_names.append(name)
        elif alloc.kind == "ExternalOutput":
            out_names.append(name)
            shape = tuple(alloc.tensor_shape)
            dtype = mybir.dt.np(alloc.dtype)
            out_avals.append(jax.core.ShapedArray(shape, dtype))
            zero_outs.append(np.zeros(shape, dtype))
    n_params = len(in_names)
    all_in_names = list(in_names) + list(out_names)
    if partition_name is not None:
        all_in_names.append(partition_name)

    def _body(*args):
        operands = list(args)
        if partition_name is not None:
            operands.append(partition_id_tensor())
        return tuple(_bass_exec_p.bind(
            *operands,
            out_avals=tuple(out_avals),
            in_names=tuple(all_in_names),
            out_names=tuple(out_names),
            lowering_input_output_aliases=(),
            sim_require_finite=True,
            sim_require_nnan=True,
            nc=nc,
        ))

    devices = jax.devices()[:N_CORES]
    mesh = Mesh(np.asarray(devices), ("core",))
    n_out = len(out_names)
    sharded = jax.jit(
        shard_map(_body, mesh=mesh,
                  in_specs=(PartitionSpec("core"),) * (n_params + n_out),
                  out_specs=(PartitionSpec("core"),) * n_out,
                  check_rep=False),
        keep_unused=True,
    )
    sh = NamedSharding(mesh, PartitionSpec("core"))
    concat_zeros = [
        np.zeros((N_CORES * z.shape[0], *z.shape[1:]), z.dtype) for z in zero_outs
    ]
    dev_zeros = [jax.device_put(a, sh) for a in concat_zeros]

    def run(in_maps):
        concat_in = [
            np.concatenate([np.asarray(in_maps[c][nm]) for c in range(N_CORES)], 0)
            for nm in in_names
        ]
        dev_in = [jax.device_put(a, sh) for a in concat_in]
        out = sharded(*dev_in, *dev_zeros)
        jax.block_until_ready(out)
        return [
            {nm: np.asarray(out[i]).reshape(N_CORES, *out_avals[i].shape)[c]
             for i, nm in enumerate(out_names)}
            for c in range(N_CORES)
        ]

    return run


def kernel(x, mask, w_qkv, b_qkv, w_out, b_out):
    x = np.asarray(x, np.float32)
    w_qkv = np.asarray(w_qkv, np.float32)
    b_qkv = np.asarray(b_qkv, np.float32)
    w_out = np.asarray(w_out, np.float32)
    b_out = np.asarray(b_out, np.float32)

    in_maps = prep_in_maps(x, mask, w_qkv, b_qkv, w_out)
    if "run" not in _COMPILED:
        _COMPILED["nc"] = build_nc()
        _COMPILED["run"] = _make_runner(_COMPILED["nc"], in_maps)
    results = _COMPILED["run"](in_maps)
    return assemble(results, b_qkv, w_out, b_out)


# revision 19
# speedup vs baseline: 2.0994x; 2.0994x over previous
"""Multi-head self-attention Bass kernel for Trainium2, 8 NeuronCores.

Problem: B=4, S=2048, D=1024, H=16 heads (dh=64), causal mask, fp32.

Sharding: core c -> batch b = c//2, head-group g = c%2 (8 heads each).
Data-parallel over B, tensor-parallel over heads; out-proj is row-parallel
with the partial-sum reduction done on the host (plus all output biases,
which fold into a single per-feature vector added on the host).

Per-core kernel (all matmuls in float32r = full-speed reduced-precision fp32):
  phase A: qkT = (Wqk x^T) [f,s layout] + bias;  V = x Wv^T [s,f layout]
  phase B: per (q-block 512, head): scoresT[k,q] = kT^T qT; P = exp(s/8);
           multiply diagonal blocks by the keep-mask; ctxT[f,q] = V_ext^T P
           with a ones column appended to V giving the softmax denominator;
           normalize via DVE multiply with a DMA-broadcast reciprocal row.
  phase C: partial_out[s,d] = ctxT^T WoT, streamed to DRAM.
Host: out[b] = partial[2b] + partial[2b+1] + (b_out + w_out @ b_v).
"""

import sys

sys.path.insert(0, "/opt/trn_rl_repo")

import ml_dtypes
import numpy as np

import concourse.bass as bass  # noqa: F401
import concourse.mybir as mybir
import concourse.tile as tile
from concourse import bacc
from concourse.bass_utils import run_bass_kernel_spmd

B = 4
S = 2048
D = 1024
H = 16
DH = 64
N_CORES = 8
HPC = H // 2          # heads per core = 8
FL = HPC * DH         # local feature width = 512
QB = 512              # q-block width
KB = 128              # k-block width
ND = D // 128         # d-blocks = 8

F32 = mybir.dt.float32
F32R = mybir.dt.float32r
BF16 = mybir.dt.bfloat16
SCALE = 1.0 / np.sqrt(DH)

_COMPILED = {}


def build_nc(s: int = S, body_reps: int = 1, diag_restrict: bool = True):
    """Build the SPMD Bass program for sequence length s. body_reps>1
    repeats the compute body (for marginal wall-clock measurement)."""
    nqb = s // QB
    nkb_all = s // KB
    nst = s // 128
    NFB = 2 * FL // 128   # 8 f-tiles of qkT (4 q + 4 k)

    nc = bacc.Bacc("TRN2", target_bir_lowering=False, debug=False,
                   num_devices=N_CORES)

    xT = nc.declare_dram_parameter("xT", [D, s], F32R, isOutput=False)
    wqkT = nc.declare_dram_parameter("wqkT", [D, 2 * FL], F32R, isOutput=False)
    wvT = nc.declare_dram_parameter("wvT", [D, FL], F32R, isOutput=False)
    b_qk = nc.declare_dram_parameter("b_qk", [128, NFB], F32, isOutput=False)
    woT = nc.declare_dram_parameter("woT", [FL, D], F32R, isOutput=False)
    band = nc.declare_dram_parameter("band", [nqb * QB, QB], BF16, isOutput=False)
    out = nc.declare_dram_parameter("out", [s, D], F32, isOutput=True)

    with tile.TileContext(nc) as tc:
        with (
            tc.tile_pool(name="persist", bufs=1) as pp,
            tc.tile_pool(name="mmpsum", bufs=2, space="PSUM") as psA,
            tc.tile_pool(name="scpsum", bufs=2, space="PSUM") as psS,
            tc.tile_pool(name="ctxpsum", bufs=2, space="PSUM") as psC,
        ):
            qkT_sb = pp.tile([128, NFB, s], F32R)
            v_ext = pp.tile([128, nst, HPC, DH + 1], F32R)
            ones_sb = pp.tile([128, HPC], F32)
            nc.vector.memset(ones_sb[:], 1.0)
            onesf = pp.tile([128, 64], F32)
            nc.vector.memset(onesf[:], 1.0)
            ones64 = pp.tile([128, 64], F32R)
            nc.vector.tensor_copy(ones64[:], onesf[:])

            for rep in range(body_reps):
                # ---------------- phase A: projections ----------------
                with tc.tile_pool(name=f"phA_{rep}", bufs=1) as pa, \
                     tc.tile_pool(name=f"wqk_{rep}", bufs=2) as pw:
                    xT_sb = pa.tile([128, ND, s], F32R, tag="xT")
                    xv = xT.ap().rearrange("(a p) s -> p a s", p=128)
                    for d in range(ND):
                        nc.sync.dma_start(xT_sb[:, d, :], xv[:, d, :])
                    wvT_sb = pa.tile([128, ND, FL], F32R, tag="wvT")
                    nc.sync.dma_start(
                        wvT_sb[:], wvT.ap().rearrange("(a p) f -> p a f", p=128))
                    bqk_sb = pa.tile([128, NFB], F32, tag="bqk")
                    nc.sync.dma_start(bqk_sb[:], b_qk.ap())

                    wqkv = wqkT.ap().rearrange("(a p) f -> p a f", p=128)
                    for fb in range(NFB):
                        wt = pw.tile([128, ND, 128], F32R, tag="wqk")
                        nc.sync.dma_start(wt[:], wqkv[:, :, 128 * fb:128 * fb + 128])
                        for sbk in range(nqb):
                            ps = psA.tile([128, QB], F32, tag="mm")
                            for d in range(ND):
                                nc.tensor.matmul(
                                    ps[:], wt[:, d, :],
                                    xT_sb[:, d, QB * sbk:QB * (sbk + 1)],
                                    start=(d == 0), stop=(d == ND - 1))
                            nc.vector.tensor_scalar_add(
                                qkT_sb[:, fb, QB * sbk:QB * (sbk + 1)], ps[:],
                                bqk_sb[:, fb:fb + 1])

                    for st in range(nst):
                        ps = psA.tile([128, FL], F32, tag="mm")
                        for d in range(ND):
                            nc.tensor.matmul(
                                ps[:], xT_sb[:, d, 128 * st:128 * (st + 1)],
                                wvT_sb[:, d, :],
                                start=(d == 0), stop=(d == ND - 1))
                        nc.vector.tensor_copy(
                            v_ext[:, st, :, 0:DH],
                            ps[:].rearrange("p (h e) -> p h e", h=HPC))
                        nc.vector.tensor_copy(v_ext[:, st, :, DH], ones_sb[:])

                # ---------------- phases B+C: attention + out-proj ----------
                with tc.tile_pool(name=f"phB_{rep}", bufs=1) as pb, \
                     tc.tile_pool(name=f"pP_{rep}", bufs=5) as pP, \
                     tc.tile_pool(name=f"pN_{rep}", bufs=2) as pN, \
                     tc.tile_pool(name=f"pctx_{rep}", bufs=2) as pctx, \
                     tc.tile_pool(name=f"pout_{rep}", bufs=2) as pout:
                    band_sb = pb.tile([128, nqb * 4, QB], BF16, tag="band")
                    nc.sync.dma_start(
                        band_sb[:],
                        band.ap().rearrange("(i k p) q -> p (i k) q", p=128, k=4))
                    woT_sb = pb.tile([128, FL // 128, D], F32R, tag="woT")
                    nc.sync.dma_start(
                        woT_sb[:], woT.ap().rearrange("(a p) d -> p a d", p=128))

                    for qb in range(nqb):
                        ctx_sb = pctx.tile([128, FL // 128, QB], F32R, tag="ctx")
                        nkb = 4 * (qb + 1)
                        for hp in range(HPC // 2):
                            # head pair (2hp, 2hp+1): h0 lives at partitions
                            # 0-63 of f-tile hp, h1 at 64-127 — the two K=64
                            # scores matmuls target disjoint PE row groups and
                            # run concurrently.
                            h0, h1 = 2 * hp, 2 * hp + 1
                            kfb = 4 + hp
                            qT0 = qkT_sb[0:64, hp, QB * qb:QB * (qb + 1)]
                            qT1 = qkT_sb[64:128, hp, QB * qb:QB * (qb + 1)]
                            pc0 = psC.tile([DH + 1, QB], F32, tag="ctxp")
                            pc1 = psC.tile([DH + 1, QB], F32, tag="ctxp")
                            for kb in range(nkb):
                                # on diagonal blocks, columns q' < off are fully
                                # masked: skip them in scores/exp/PV, and apply
                                # the keep-mask only on the mixed 128-wide square.
                                diag = kb >= 4 * qb
                                off = (128 * (kb - 4 * qb)
                                       if (diag and diag_restrict) else 0)
                                ps = psS.tile([128, 2, QB], F32, tag="sc")
                                kcol = slice(128 * kb, 128 * (kb + 1))
                                nc.tensor.matmul(
                                    ps[:, 0, off:], qkT_sb[0:64, kfb, kcol],
                                    qT0[:, off:], start=True, stop=True)
                                nc.tensor.matmul(
                                    ps[:, 1, off:], qkT_sb[64:128, kfb, kcol],
                                    qT1[:, off:], start=True, stop=True)
                                pt = pP.tile([128, 2, QB], F32R, tag="P")
                                nc.scalar.activation(
                                    pt[:, :, off:], ps[:, :, off:],
                                    mybir.ActivationFunctionType.Exp,
                                    scale=float(SCALE))
                                if diag:
                                    # only the 128-wide square straddling the
                                    # diagonal is mixed; under diag_restrict
                                    # the fully-masked columns are never
                                    # computed at all.
                                    mo = 128 * (kb - 4 * qb)
                                    lo = mo if diag_restrict else 0
                                    nc.vector.tensor_mul(
                                        pt[:, :, lo:mo + 128],
                                        pt[:, :, lo:mo + 128],
                                        band_sb[:, 4 * qb + (kb - 4 * qb), None,
                                                lo:mo + 128]
                                        .to_broadcast((128, 2, mo + 128 - lo)))
                                nc.tensor.matmul(
                                    pc0[:, off:], v_ext[:, kb, h0, :],
                                    pt[:, 0, off:],
                                    start=(kb == 0), stop=(kb == nkb - 1))
                                nc.tensor.matmul(
                                    pc1[:, off:], v_ext[:, kb, h1, :],
                                    pt[:, 1, off:],
                                    start=(kb == 0), stop=(kb == nkb - 1))
                            # normalize: reciprocal of the denominator row,
                            # PE ones-outer-product broadcast, multiply.
                            for j, pc in ((0, pc0), (1, pc1)):
                                rec = pN.tile([128, QB], F32R, tag="rec")
                                with nc.allow_low_precision(
                                        reason="f32r recip feeds PE broadcast"):
                                    nc.vector.reciprocal(rec[64:65, :],
                                                         pc[DH:DH + 1, :])
                                pbc = psA.tile([64, QB], F32, tag="mm")
                                nc.tensor.matmul(
                                    pbc[:], ones64[64:65, :], rec[64:65, :],
                                    start=True, stop=True)
                                bc = pN.tile([64, QB], F32, tag="bc")
                                nc.scalar.activation(
                                    bc[:], pbc[:],
                                    mybir.ActivationFunctionType.Copy)
                                if j == 0:
                                    nc.vector.tensor_mul(
                                        ctx_sb[0:64, hp, :], pc[0:DH, :], bc[:])
                                else:
                                    tmp = pN.tile([64, QB], F32R, tag="tmp")
                                    nc.vector.tensor_mul(tmp[:], pc[0:DH, :],
                                                         bc[:])
                                    nc.sync.dma_start(ctx_sb[64:128, hp, :],
                                                      tmp[:])

                        # out-proj for this q-block
                        for st4 in range(QB // 128):
                            ot = pout.tile([128, D], F32, tag="ot")
                            for dh2 in range(2):
                                po = psA.tile([128, 512], F32, tag="mm")
                                for fb2 in range(FL // 128):
                                    nc.tensor.matmul(
                                        po[:],
                                        ctx_sb[:, fb2, 128 * st4:128 * (st4 + 1)],
                                        woT_sb[:, fb2, 512 * dh2:512 * (dh2 + 1)],
                                        start=(fb2 == 0), stop=(fb2 == 3))
                                nc.vector.tensor_copy(
                                    ot[:, 512 * dh2:512 * (dh2 + 1)], po[:])
                            st = 4 * qb + st4
                            nc.sync.dma_start(
                                out[128 * st:128 * (st + 1), :], ot[:])
    nc.finalize()
    return nc


def prep_in_maps(x, mask, w_qkv, b_qkv, w_out, s: int = S):
    nqb = s // QB
    m = np.asarray(mask)[0, 0]
    band = np.empty((nqb * QB, QB), ml_dtypes.bfloat16)
    for i in range(nqb):
        band[QB * i:QB * (i + 1)] = (~m[QB * i:QB * (i + 1),
                                        QB * i:QB * (i + 1)]).T
    in_maps = []
    for c in range(N_CORES):
        b, g = c // 2, c % 2
        wq = w_qkv[FL * g:FL * (g + 1)]
        wk = w_qkv[D + FL * g:D + FL * (g + 1)]
        wv = w_qkv[2 * D + FL * g:2 * D + FL * (g + 1)]
        bq = b_qkv[FL * g:FL * (g + 1)]
        bk = b_qkv[D + FL * g:D + FL * (g + 1)]
        bqk = np.concatenate([bq, bk]).reshape(2 * FL // 128, 128).T
        in_maps.append({
            "xT": np.ascontiguousarray(x[b].T),
            "wqkT": np.ascontiguousarray(np.concatenate([wq, wk], 0).T),
            "wvT": np.ascontiguousarray(wv.T),
            "b_qk": np.ascontiguousarray(bqk),
            "woT": np.ascontiguousarray(w_out[:, FL * g:FL * (g + 1)].T),
            "band": band,
        })
    return in_maps


def assemble(results, b_qkv, w_out, b_out, s: int = S):
    bv = b_qkv[2 * D:]
    bias_full = b_out + w_out @ bv
    outp = np.empty((B, s, D), np.float32)
    for b in range(B):
        outp[b] = (results[2 * b]["out"] + results[2 * b + 1]["out"]
                   + bias_full[None, :])
    return outp


def _make_runner(nc, in_maps):
    """Persistent PJRT runner: trace/compile once, reuse the executable."""
    import jax
    from jax.sharding import Mesh, PartitionSpec, NamedSharding
    from jax.experimental.shard_map import shard_map
    from concourse import bass2jax
    from concourse.bass2jax import _bass_exec_p, partition_id_tensor

    bass2jax.install_neuronx_cc_hook()
    partition_name = nc.partition_id_tensor.name if nc.partition_id_tensor else None
    in_names, out_names, out_avals, zero_outs = [], [], [], []
    for alloc in nc.m.functions[0].allocations:
        if not isinstance(alloc, mybir.MemoryLocationSet):
            continue
        name = alloc.memorylocations[0].name
        if alloc.kind == "ExternalInput":
            if name != partition_name:
                in_names.append(name)
        elif alloc.kind == "ExternalOutput":
            out_names.append(name)
            shape = tuple(alloc.tensor_shape)
            dtype = mybir.dt.np(alloc.dtype)
            out_avals.append(jax.core.ShapedArray(shape, dtype))
            zero_outs.append(np.zeros(shape, dtype))
    n_params = len(in_names)
    all_in_names = list(in_names) + list(out_names)
    if partition_name is not None:
        all_in_names.append(partition_name)

    def _body(*args):
        operands = list(args)
        if partition_name is not None:
            operands.append(partition_id_tensor())
        return tuple(_bass_exec_p.bind(
            *operands,
            out_avals=tuple(out_avals),
            in_names=tuple(all_in_names),
            out_names=tuple(out_names),
            lowering_input_output_aliases=(),
            sim_require_finite=True,
            sim_require_nnan=True,
            nc=nc,
        ))

    devices = jax.devices()[:N_CORES]
    mesh = Mesh(np.asarray(devices), ("core",))
    n_out = len(out_names)
    sharded = jax.jit(
        shard_map(_body, mesh=mesh,
                  in_specs=(PartitionSpec("core"),) * (n_params + n_out),
                  out_specs=(PartitionSpec("core"),) * n_out,
                  check_rep=False),
        keep_unused=True,
    )
    sh = NamedSharding(mesh, PartitionSpec("core"))
    concat_zeros = [
        np.zeros((N_CORES * z.shape[0], *z.shape[1:]), z.dtype) for z in zero_outs
    ]
    dev_zeros = [jax.device_put(a, sh) for a in concat_zeros]

    def run(in_maps):
        concat_in = [
            np.concatenate([np.asarray(in_maps[c][nm]) for c in range(N_CORES)], 0)
            for nm in in_names
        ]
        dev_in = [jax.device_put(a, sh) for a in concat_in]
        out = sharded(*dev_in, *dev_zeros)
        jax.block_until_ready(out)
        return [
            {nm: np.asarray(out[i]).reshape(N_CORES, *out_avals[i].shape)[c]
             for i, nm in enumerate(out_names)}
            for c in range(N_CORES)
        ]

    return run


def kernel(x, mask, w_qkv, b_qkv, w_out, b_out):
    x = np.asarray(x, np.float32)
    w_qkv = np.asarray(w_qkv, np.float32)
    b_qkv = np.asarray(b_qkv, np.float32)
    w_out = np.asarray(w_out, np.float32)
    b_out = np.asarray(b_out, np.float32)

    in_maps = prep_in_maps(x, mask, w_qkv, b_qkv, w_out)
    if "run" not in _COMPILED:
        _COMPILED["nc"] = build_nc()
        _COMPILED["run"] = _make_runner(_COMPILED["nc"], in_maps)
    results = _COMPILED["run"](in_maps)
    return assemble(results, b_qkv, w_out, b_out)
